# revision 3
# baseline (speedup 1.0000x reference)
"""Neural CDE Trainium2 kernel, v3 — P1Y-state + streamed Z + fp32r.

Differences vs v2:
  - Z (dX broadcast to 128 partitions) is precomputed on host and DMA-streamed
    in double-buffered chunks; no on-device dxt/ebc expansion.
  - Single rZu per stage: bank_s uses ratio-scaled W0G blocks
    (w0gc3 = 3x, w0gc15 = 1.5x base) so rZc is not materialized.
  - eye-mm for the next stage's bank_s is emitted before bank_y's W0G mms
    so it executes during the ACT exp-chain.
  - Optional float32r matmul operands (F32R): 2 cycles/row instead of 4.
"""

import numpy as np

B, T, D, H, W = 128, 1024, 8, 64, 128
NCORES = 8
BS = B // NCORES
NSTEPS_FULL = T - 1
CH = 128                       # steps per Z chunk

F32R = False

_SROW = (0, 1, 1, 2)
_CJ = (0.0, 0.5, 0.5, 1.0)
_UJ = (1.0 / 6.0, 1.0 / 3.0, 1.0 / 3.0, 1.0 / 6.0)
# bank_s at stage j (>=1) uses rZu_{j-1} scaled by ratio = c_j / u_{j-1}
_RATIO = (None, 3.0, 1.5, 3.0)

_L = {}
_off = 0
for _name, _p, _f in [
    ("fw1p", W, W), ("fw2p", W, 512),
    ("w0g", 128, 4 * W),       # -2 * W0G (for bank_y, with u_j in rZu)
    ("w0gc3", 128, 4 * W),     # 3x base  (bank_s stages 1,3)
    ("w0gc15", 128, 4 * W),    # 1.5x base (bank_s stage 2)
    ("b3l", 4, 128), ("b3r", 4, 4 * BS),
]:
    _L[_name] = (_p, _off, _f)
    _off += _f
WMM_F = _off

# fp32 matmul constants: the state path (eye-mm, init MLP, readout) must not
# be TF32-rounded — eye-mm re-rounds the whole state every step, and the
# pinv readout is cancellation-heavy.
_LF = {}
_offf = 0
for _name, _p, _f in [
    ("eye", 128, 128),
    ("fw0p", H, W),
    ("iw0p", D, W), ("iw1p", W, W), ("iw2p", W, H),
    ("x0T", D, BS), ("lwPT", W, 1),
]:
    _LF[_name] = (_p, _offf, _f)
    _offf += _f
WF32_F = _offf

_LA = {}
_offa = 0
for _name, _p, _f in [
    ("ib0", W, 1), ("ib1", W, 1), ("ib2", H, 1),
    ("fb0", W, 1), ("fb1", W, 1), ("lbneg", 1, 1),
]:
    _LA[_name] = (_p, _offa, _f)
    _offa += _f
WACT_F = _offa


def build_bass(nsteps):
    import concourse.bass as bass
    import concourse.bacc as bacc
    import concourse.mybir as mybir
    from concourse import tile

    f32 = mybir.dt.float32
    fmm = mybir.dt.float32r if F32R else f32
    AF = mybir.ActivationFunctionType
    ALU = mybir.AluOpType

    nch = (nsteps + CH - 1) // CH

    nc = bacc.Bacc(None)

    wmm_d = nc.declare_dram_parameter("wmm", [128, WMM_F], fmm, isOutput=False)
    wf32_d = nc.declare_dram_parameter("wf32", [128, WF32_F], f32, isOutput=False)
    wact_d = nc.declare_dram_parameter("wact", [128, WACT_F], f32, isOutput=False)
    zbs_d = [
        nc.declare_dram_parameter(f"zbs{s}", [128, nsteps * BS], f32, isOutput=False)
        for s in range(3)
    ]
    out_d = nc.declare_dram_parameter("out", [1, BS], f32, isOutput=True)

    with tile.TileContext(nc) as tc:
        with (
            tc.tile_pool(name="const", bufs=1) as cpool,
            tc.tile_pool(name="zch", bufs=2) as zpool,
            tc.tile_pool(name="state", bufs=1) as spool,
            tc.tile_pool(name="work", bufs=2) as wk,
            tc.tile_pool(name="work64", bufs=2) as w64,
            tc.tile_pool(name="ps_s", bufs=2, space="PSUM") as ps_s,
            tc.tile_pool(name="ps_p2", bufs=2, space="PSUM") as ps_p2,
            tc.tile_pool(name="ps_p3", bufs=2, space="PSUM") as ps_p3,
            tc.tile_pool(name="ps_y", bufs=1, space="PSUM") as ps_y,
        ):
            wmm = cpool.tile([128, WMM_F], fmm, tag="wmm")
            nc.sync.dma_start(wmm[:], wmm_d[:])
            wf = cpool.tile([128, WF32_F], f32, tag="wf")
            nc.sync.dma_start(wf[:], wf32_d[:])
            wact = cpool.tile([128, WACT_F], f32, tag="wact")
            nc.sync.dma_start(wact[:], wact_d[:])

            def C(name):
                if name in _LF:
                    p, o, f = _LF[name]
                    return wf[0:p, o : o + f]
                p, o, f = _L[name]
                return wmm[0:p, o : o + f]

            def CA(name):
                p, o, f = _LA[name]
                return wact[0:p, o : o + f]

            warm = wk.tile([1, 4], f32, tag="warm")
            nc.scalar.activation(warm[0:1, 0:1], wact[0:1, 0:1], AF.Copy)
            nc.vector.tensor_copy(warm[0:1, 1:2], wact[0:1, 0:1])

            # Z chunks: zc[k % 2][v] tiles, DMA'd ahead
            zc_tiles = []
            def dma_chunk(k):
                tiles = []
                for v in range(3):
                    zt = zpool.tile([128, CH * BS], f32, name=f"zc{v}_{k}",
                                    tag=f"zc{v}")
                    lo = k * CH * BS
                    hi = min(nsteps * BS, (k + 1) * CH * BS)
                    nc.sync.dma_start(zt[:, 0 : hi - lo], zbs_d[v][:, lo:hi])
                    tiles.append(zt)
                zc_tiles.append(tiles)
                for v in range(3):
                    nc.vector.tensor_copy(warm[0:1, 2:3], tiles[v][0:1, 0:1])

            dma_chunk(0)
            if nch > 1:
                dma_chunk(1)

            P1Y = spool.tile([W, BS], f32, tag="P1Y")
            racc = spool.tile([128, 1], f32, tag="racc")

            # ---- init ----
            pi = ps_s.tile([W, BS], f32, tag="ps_s")
            nc.tensor.matmul(pi[:], C("iw0p"), C("x0T"), start=True, stop=True)
            h1 = wk.tile([W, BS], f32, tag="h1")
            nc.scalar.activation(h1[:], pi[:], AF.Relu, bias=CA("ib0"))
            pi2 = ps_p2.tile([W, BS], f32, tag="ps_p2")
            nc.tensor.matmul(pi2[:], C("iw1p"), h1[:], start=True, stop=True)
            h2 = wk.tile([W, BS], f32, tag="h2")
            nc.scalar.activation(h2[:], pi2[:], AF.Relu, bias=CA("ib1"))
            pk = ps_y.tile([W, BS], f32, tag="ps_y")
            nc.tensor.matmul(pk[0:H, :], C("iw2p"), h2[:], start=True, stop=True)
            y0 = wk.tile([H, BS], f32, tag="y0")
            nc.scalar.activation(y0[:], pk[0:H, :], AF.Identity, bias=CA("ib2"))
            pp = ps_s.tile([W, BS], f32, tag="ps_s")
            nc.tensor.matmul(pp[:], C("fw0p"), y0[:], start=True, stop=True)
            nc.scalar.activation(P1Y[:], pp[:], AF.Copy)

            # ---- scan ----
            bank_s_next = None
            for t in range(nsteps):
                k = t // CH
                off = (t % CH) * BS
                if t % CH == 0 and t > 0 and k + 1 < nch:
                    dma_chunk(k + 1)
                zck = zc_tiles[k]

                bank_y = ps_y.tile([W, BS], f32, tag="ps_y")
                rZ_prev = None
                for j in range(4):
                    zb = zck[_SROW[j]]

                    if j == 0:
                        p1src = P1Y
                        nc.tensor.matmul(bank_y[:], C("eye"), P1Y[:],
                                         start=True, stop=False)
                    else:
                        bank_s = bank_s_next
                        wg = C("w0gc3") if _RATIO[j] == 3.0 else C("w0gc15")
                        for c in range(4):
                            nc.tensor.matmul(
                                bank_s[:], wg[:, c * W : (c + 1) * W],
                                rZ_prev[:, c, :],
                                start=False, stop=(c == 3),
                            )
                        p1src = bank_s

                    u1 = wk.tile([W, BS], f32, tag="u1")
                    nc.scalar.activation(u1[:], p1src[:], AF.Exp, bias=CA("fb0"))
                    s1 = wk.tile([W, BS], fmm, tag="s1")
                    nc.scalar.activation(s1[:], u1[:], AF.Ln, bias=1.0)

                    p2 = ps_p2.tile([W, BS], f32, tag="ps_p2")
                    nc.tensor.matmul(p2[:], C("fw1p"), s1[:], start=True, stop=True)
                    u2 = wk.tile([W, BS], f32, tag="u2")
                    nc.scalar.activation(u2[:], p2[:], AF.Exp, bias=CA("fb1"))
                    s2 = wk.tile([W, BS], fmm, tag="s2")
                    nc.scalar.activation(s2[:], u2[:], AF.Ln, bias=1.0)

                    p3 = ps_p3.tile([128, 4 * BS], f32, tag="ps_p3")
                    nc.tensor.matmul(p3[:], C("b3l"), C("b3r"), start=True, stop=False)
                    fw2p = C("fw2p")
                    for c in range(4):
                        nc.tensor.matmul(
                            p3[:, c * BS : (c + 1) * BS],
                            fw2p[:, c * 128 : (c + 1) * 128],
                            s2[:],
                            start=False, stop=(c == 3),
                        )

                    # next bank_s init rides here (executes during ACT chain)
                    if j < 3:
                        bank_s_next = ps_s.tile([W, BS], f32, tag="ps_s")
                        nc.tensor.matmul(bank_s_next[:], C("eye"), P1Y[:],
                                         start=True, stop=False)

                    texp = w64.tile([128, 4 * BS], f32, tag="texp")
                    nc.scalar.activation(texp[:], p3[:], AF.Exp, scale=2.0)
                    den = w64.tile([128, 4 * BS], f32, tag="den")
                    nc.vector.tensor_scalar(
                        den[:], texp[:], 1.0e30, 1.0, ALU.min, ALU.add
                    )
                    r = w64.tile([128, 4 * BS], f32, tag="r")
                    nc.vector.reciprocal_approx_fast(r[:], den[:])

                    r3 = r[:, :]
                    r3 = bass.AP(
                        r3.tensor, r3.offset,
                        [r3.ap[0], [BS, 4], [1, BS]],
                    )
                    zb_s = zb[:, off : off + BS]
                    zb_b = bass.AP(
                        zb_s.tensor, zb_s.offset,
                        [zb_s.ap[0], [0, 4], zb_s.ap[1]],
                    )
                    uj = _UJ[j]
                    rZ = w64.tile([128, 4, BS], fmm, tag="rZ")
                    nc.vector.affine_mul_reduce(
                        rZ[:], racc[:], r3, zb_b, uj, -0.5 * uj
                    )

                    wgu = C("w0g")
                    for c in range(4):
                        nc.tensor.matmul(
                            bank_y[:], wgu[:, c * W : (c + 1) * W],
                            rZ[:, c, :],
                            start=False, stop=(j == 3 and c == 3),
                        )
                    rZ_prev = rZ

                nc.scalar.activation(P1Y[:], bank_y[:], AF.Copy)

            # ---- readout ----
            pr = ps_s.tile([W, BS], f32, tag="ps_s")
            nc.tensor.matmul(pr[0:1, :], C("lwPT"), P1Y[:], start=True, stop=True)
            er = wk.tile([1, BS], f32, tag="er")
            nc.scalar.activation(er[:], pr[0:1, :], AF.Exp, bias=CA("lbneg"), scale=-1.0)
            dr = wk.tile([1, BS], f32, tag="dr")
            nc.vector.tensor_scalar_add(dr[:], er[:], 1.0)
            rr = wk.tile([1, BS], f32, tag="rr")
            nc.vector.reciprocal(rr[:], dr[:])
            nc.sync.dma_start(out_d[:], rr[:])

    nc.compile()
    return nc


def prep_inputs(ts, coeff_d, coeff_c, coeff_b, coeff_a,
                iw0, ib0, iw1, ib1, iw2, ib2,
                fw0, fb0, fw1, fb1, fw2, fb2, lw, lb, nsteps=NSTEPS_FULL):
    f = np.float32
    cd = np.asarray(coeff_d, f)[:, :nsteps, :]
    cc = np.asarray(coeff_c, f)[:, :nsteps, :]
    cb = np.asarray(coeff_b, f)[:, :nsteps, :]
    ca = np.asarray(coeff_a, f)

    dXs = [cb, (0.75 * cd + cc + cb).astype(f), (3.0 * cd + 2.0 * cc + cb).astype(f)]

    fw0 = np.asarray(fw0, f)
    fw2 = np.asarray(fw2, f)
    fb2 = np.asarray(fb2, f)

    def fillL(buf, L, name, arr):
        p, o, fl = L[name]
        assert arr.shape == (p, fl), (name, arr.shape, (p, fl))
        buf[0:p, o : o + fl] = arr

    wmm0 = np.zeros((128, WMM_F), f)
    wf0 = np.zeros((128, WF32_F), f)
    def fill(n, a):
        if n in _LF:
            fillL(wf0, _LF, n, a)
        else:
            fillL(wmm0, _L, n, a)
    fill("fw0p", np.ascontiguousarray(fw0.T))
    fill("fw1p", np.ascontiguousarray(np.asarray(fw1, f).T))

    fw2p = np.zeros((W, 512), f)
    b3l = np.zeros((4, 128), f)
    for c in range(4):
        for p in range(128):
            h = 16 * c + (p % 16)
            d = p // 16
            fw2p[:, c * 128 + p] = fw2[h * D + d, :]
            b3l[c, p] = fb2[h * D + d]
    fill("fw2p", fw2p)
    fill("b3l", b3l)
    b3r = np.zeros((4, 4 * BS), f)
    for c in range(4):
        b3r[c, c * BS : (c + 1) * BS] = 1.0
    fill("b3r", b3r)

    def w0g(sigma):
        out = np.zeros((128, 4 * W), f)
        for c in range(4):
            for p in range(128):
                out[p, c * W : (c + 1) * W] = sigma * fw0[:, 16 * c + (p % 16)]
        return out

    fill("w0g", w0g(-2.0))
    fill("w0gc3", w0g(-2.0 * 3.0))
    fill("w0gc15", w0g(-2.0 * 1.5))
    fill("eye", np.eye(128, dtype=f))

    fill("iw0p", np.ascontiguousarray(np.asarray(iw0, f).T))
    fill("iw1p", np.ascontiguousarray(np.asarray(iw1, f).T))
    fill("iw2p", np.ascontiguousarray(np.asarray(iw2, f).T))

    lwP = (np.asarray(lw, np.float64) @ np.linalg.pinv(np.asarray(fw0, np.float64)))
    fill("lwPT", np.ascontiguousarray(lwP.astype(f).reshape(1, W).T))

    wact0 = np.zeros((128, WACT_F), f)
    filla = lambda n, a: fillL(wact0, _LA, n, a)
    filla("ib0", np.asarray(ib0, f)[:, None])
    filla("ib1", np.asarray(ib1, f)[:, None])
    filla("ib2", np.asarray(ib2, f)[:, None])
    filla("fb0", np.asarray(fb0, f)[:, None])
    filla("fb1", np.asarray(fb1, f)[:, None])
    filla("lbneg", -np.asarray(lb, f).reshape(1, 1))

    in_maps = []
    for i in range(NCORES):
        sl = slice(i * BS, (i + 1) * BS)
        wfb = wf0.copy()
        fillL(wfb, _LF, "x0T", np.ascontiguousarray(ca[sl, 0, :].T))
        m = {"wmm": wmm0, "wf32": wfb, "wact": wact0}
        for v in range(3):
            arr = dXs[v][sl].transpose(2, 1, 0)        # [8, nsteps, 16]
            zb = np.repeat(arr, 16, axis=0)            # [128, nsteps, 16]
            m[f"zbs{v}"] = np.ascontiguousarray(zb.reshape(128, -1))
        in_maps.append(m)
    return in_maps


_CACHE = {}


def _get_nc(nsteps):
    if nsteps not in _CACHE:
        _CACHE[nsteps] = build_bass(nsteps)
    return _CACHE[nsteps]


def kernel(**inputs):
    from concourse.bass_utils import run_bass_kernel_spmd

    nsteps = NSTEPS_FULL
    in_maps = prep_inputs(nsteps=nsteps, **inputs)
    nc = _get_nc(nsteps)
    res = run_bass_kernel_spmd(nc, in_maps, list(range(NCORES)))
    outs = [res.results[i]["out"].reshape(BS) for i in range(NCORES)]
    return np.concatenate(outs, axis=0).astype(np.float32)
